# revision 14
# baseline (speedup 1.0000x reference)
"""Trainium2 Bass kernel for nn_Experts (GIN message passing + 4 expert branches).

Parallelization (8 cores, single SPMD launch, uniform program, per-core data):
  Phase A  shared 3-layer GIN encoder, node-sharded 8 ways; segment-sum by
           one-hot matmul over dst-sorted edge tiles; AllGather h per layer.
  Phase B  fused 4-expert edge-mask MLP ([256->512]->relu->[512->4]) over
           edge shards; hard gumbel gate as (em_logits + g) > 0 (exact
           forward of the straight-through estimator); AllGather e_on.
  nw       endpoint-doubled scatter-count via one-hot matmul; AllGather nw.
  Phase C  per-expert masked GIN: cores (2k, 2k+1) own expert k, each half
           the node blocks; pair AllGather between layers; mean-pool via
           batch-one-hot matmul; pair AllReduce of pooled sums; head MLP.
Everything fp32. Segment sums are exact (0/1 one-hot, fp32 PSUM accum).
"""

import sys
import numpy as np

sys.path.insert(0, "/opt/trn_rl_repo")

from concourse import bass, bacc, mybir, tile  # noqa: E402
from concourse import bass_utils  # noqa: E402
from concourse.masks import make_identity  # noqa: E402

F32 = mybir.dt.float32
I32 = mybir.dt.int32
AF = mybir.ActivationFunctionType
OP = mybir.AluOpType

N, E, F, H, K, L, B, C = 20000, 320000, 128, 128, 4, 3, 64, 2
P = 128
NC8 = 8
NBLK = 160
NPAD = NBLK * P
BPC = NBLK // NC8          # 20 blocks per core (phases A/B/nw)
BPH = NBLK // 2            # 80 blocks per half (phase C)
G8 = [list(range(8))]
G2 = [[0, 1], [2, 3], [4, 5], [6, 7]]
EPS20 = 1e-20


# ======================================================================
# host preprocessing
# ======================================================================

def _runs_to_slots(order, blk_of_edge, nblocks, tmax):
    slots = np.full(nblocks * tmax * P, -1, dtype=np.int64)
    blk = blk_of_edge[order]
    bounds = np.searchsorted(blk, np.arange(nblocks + 1))
    for b in range(nblocks):
        lo, hi = bounds[b], bounds[b + 1]
        slots[b * tmax * P: b * tmax * P + (hi - lo)] = order[lo:hi]
    return slots


def _blockify(arr_slot, nblocks, tmax):
    """[nblocks*tmax*P, ...] slot-linear -> [nblocks*P, tmax] with
    [b*P+p, t] = value at slot (b*tmax+t)*P + p."""
    a = arr_slot.reshape(nblocks, tmax, P)
    return np.ascontiguousarray(a.transpose(0, 2, 1)).reshape(nblocks * P, tmax)


def _host_prep(inputs):
    x = np.asarray(inputs["x"], np.float32)
    ei = np.asarray(inputs["edge_index"]).astype(np.int64)
    batch = np.asarray(inputs["batch"]).astype(np.int64)
    u = np.asarray(inputs["u"], np.float32)
    src, dst = ei[0], ei[1]

    order_d = np.argsort(dst, kind="stable")
    cnt_d = np.bincount(dst // P, minlength=NBLK)
    Tmax = int(np.ceil(cnt_d.max() / P))
    slots = _runs_to_slots(order_d, dst // P, NBLK, Tmax)
    NSLOT = slots.size
    valid = slots >= 0
    e_of = np.where(valid, slots, 0)
    src_slot = np.where(valid, src[e_of], 0).astype(np.int32)
    dst_slot = np.where(valid, dst[e_of], 0).astype(np.int32)
    dstl_slot = np.where(valid, dst[e_of] % P, -1).astype(np.float32)
    uT_slot = np.full((4, NSLOT), 0.5, np.float32)
    uT_slot[:, valid] = u[e_of[valid]].T
    slot_of_edge = np.zeros(E, np.int64)
    slot_of_edge[slots[valid]] = np.nonzero(valid)[0]

    ep_nodes = np.concatenate([src, dst])
    ep_edge = np.concatenate([np.arange(E), np.arange(E)])
    order_e = np.argsort(ep_nodes, kind="stable")
    cnt_e = np.bincount(ep_nodes // P, minlength=NBLK)
    Tnw = int(np.ceil(cnt_e.max() / P))
    eslots = _runs_to_slots(order_e, ep_nodes // P, NBLK, Tnw)
    evalid = eslots >= 0
    ee = np.where(evalid, eslots, 0)
    eid_slot = np.where(evalid, slot_of_edge[ep_edge[ee]], 0).astype(np.int32)
    epl_slot = np.where(evalid, ep_nodes[ee] % P, -1).astype(np.float32)

    batch_pad = np.full(NPAD, 127.0, np.float32)
    batch_pad[:N] = batch.astype(np.float32)
    cnt_b = np.bincount(batch, minlength=B).astype(np.float32)
    inv_cnt = (1.0 / np.maximum(cnt_b, 1.0)).astype(np.float32)
    x_pad = np.zeros((NPAD, F), np.float32)
    x_pad[:N] = x

    return dict(Tmax=Tmax, Tnw=Tnw, NSLOT=NSLOT, slot_of_edge=slot_of_edge,
                src_slot=src_slot, dst_slot=dst_slot, dstl_slot=dstl_slot,
                uT_slot=uT_slot, eid_slot=eid_slot, epl_slot=epl_slot,
                batch_pad=batch_pad, inv_cnt=inv_cnt, x_pad=x_pad)


def _per_core_inputs(pp, inputs):
    Tmax, Tnw = pp["Tmax"], pp["Tnw"]
    SPC = BPC * Tmax * P               # slots per core
    HS = BPH * Tmax * P                # slots per half
    ESPC = BPC * Tnw * P

    src_blk = _blockify(pp["src_slot"], NBLK, Tmax)      # [NBLK*P, Tmax]
    dstl_blk = _blockify(pp["dstl_slot"], NBLK, Tmax)
    eid_blk = _blockify(pp["eid_slot"], NBLK, Tnw)
    epl_blk = _blockify(pp["epl_slot"], NBLK, Tnw)

    enc_scale = np.repeat((1.0 + np.asarray(inputs["enc_eps"], np.float32))[:, None], P, 1)

    def wcat(W):       # [L,H,H] -> [H, L*H]
        W = np.asarray(W, np.float32)
        return np.concatenate([W[l] for l in range(W.shape[0])], axis=1)

    mW1 = np.asarray(inputs["mask_W1"], np.float32)      # [K, 2H, H]
    mW1a = np.concatenate([mW1[k, :H, :] for k in range(K)], axis=1)   # [128, 512]
    mW1b = np.concatenate([mW1[k, H:, :] for k in range(K)], axis=1)
    mb1 = np.asarray(inputs["mask_b1"], np.float32).T                  # [128,4]
    mW2 = np.asarray(inputs["mask_W2"], np.float32)      # [K, H, 1]
    mW2r = np.zeros((H, 4 * K), np.float32)              # slice hs -> [128,4]
    for k in range(K):
        mW2r[:, k * 4 + k] = mW2[k, :, 0]
    mb2 = np.asarray(inputs["mask_b2"], np.float32).reshape(K, 1)      # [4,1]

    maps = []
    for c in range(NC8):
        k = c // 2
        half = c % 2
        cls_scale = np.repeat(
            (1.0 + np.asarray(inputs["cls_eps"], np.float32)[k])[:, None], P, 1)
        sel = np.zeros((P, 4), np.float32)
        sel[:, k] = 1.0
        m = {
            "x_pad": pp["x_pad"],
            "src_blk": src_blk[c * BPC * P:(c + 1) * BPC * P],
            "dstl_blk": dstl_blk[c * BPC * P:(c + 1) * BPC * P],
            "srcB": pp["src_slot"][c * SPC:(c + 1) * SPC][:, None],
            "dstB": pp["dst_slot"][c * SPC:(c + 1) * SPC][:, None],
            "uT": np.ascontiguousarray(pp["uT_slot"][:, c * SPC:(c + 1) * SPC]),
            "eid_blk": eid_blk[c * BPC * P:(c + 1) * BPC * P],
            "epl_blk": epl_blk[c * BPC * P:(c + 1) * BPC * P],
            "src_half": src_blk[half * BPH * P:(half + 1) * BPH * P],
            "dstl_half": dstl_blk[half * BPH * P:(half + 1) * BPH * P],
            "cslot": np.arange(half * HS, (half + 1) * HS, dtype=np.int32)[:, None],
            "own_nodes": np.arange(c * BPC * P, (c + 1) * BPC * P,
                                   dtype=np.int32)[:, None],
            "own_half": np.arange(half * BPH * P, (half + 1) * BPH * P,
                                  dtype=np.int32)[:, None],
            "batchl": pp["batch_pad"][:, None],
            "batchl_half": pp["batch_pad"][half * BPH * P:(half + 1) * BPH * P][:, None],
            "inv_cnt_col": pp["inv_cnt"][:, None],
            "encW1": wcat(inputs["enc_W1"]), "encW2": wcat(inputs["enc_W2"]),
            "encb1": np.asarray(inputs["enc_b1"], np.float32),
            "encb2": np.asarray(inputs["enc_b2"], np.float32),
            "enc_scale": enc_scale,
            "clsW1": wcat(np.asarray(inputs["cls_W1"], np.float32)[k]),
            "clsW2": wcat(np.asarray(inputs["cls_W2"], np.float32)[k]),
            "clsb1": np.asarray(inputs["cls_b1"], np.float32)[k],
            "clsb2": np.asarray(inputs["cls_b2"], np.float32)[k],
            "cls_scale": cls_scale,
            "mW1a": mW1a, "mW1b": mW1b, "mb1": mb1, "mW2r": mW2r, "mb2": mb2,
            "headW1": np.asarray(inputs["head_W1"], np.float32)[k],
            "headb1": np.asarray(inputs["head_b1"], np.float32)[k][:, None],
            "headW2": np.asarray(inputs["head_W2"], np.float32)[k],
            "headb2": np.asarray(inputs["head_b2"], np.float32)[k][:, None],
            "sel": sel,
        }
        maps.append({kk: np.ascontiguousarray(vv) for kk, vv in m.items()})
    return maps


# ======================================================================
# device program
# ======================================================================

def _build(Tmax, Tnw):
    SPC = BPC * Tmax * P
    HS = BPH * Tmax * P
    NSLOT = NBLK * Tmax * P
    NTILE_H = HS // P          # phase-C slot tiles per half

    nc = bacc.Bacc("TRN2", target_bir_lowering=False, debug=False,
                   num_devices=NC8)

    def din(name, shape, dt=F32):
        return nc.dram_tensor(name, shape, dt, kind="ExternalInput")

    x_t = din("x_pad", [NPAD, F])
    src_blk = din("src_blk", [BPC * P, Tmax], I32)
    dstl_blk = din("dstl_blk", [BPC * P, Tmax])
    srcB = din("srcB", [SPC, 1], I32)
    dstB = din("dstB", [SPC, 1], I32)
    uT = din("uT", [4, SPC])
    eid_blk = din("eid_blk", [BPC * P, Tnw], I32)
    epl_blk = din("epl_blk", [BPC * P, Tnw])
    src_half = din("src_half", [BPH * P, Tmax], I32)
    dstl_half = din("dstl_half", [BPH * P, Tmax])
    cslot = din("cslot", [HS, 1], I32)
    own_nodes = din("own_nodes", [BPC * P, 1], I32)
    own_half = din("own_half", [BPH * P, 1], I32)
    batchl = din("batchl", [NPAD, 1])
    batchl_half = din("batchl_half", [BPH * P, 1])
    inv_cnt_col = din("inv_cnt_col", [B, 1])
    encW1 = din("encW1", [H, L * H]); encW2 = din("encW2", [H, L * H])
    encb1 = din("encb1", [L, H]); encb2 = din("encb2", [L, H])
    enc_scale = din("enc_scale", [L, P])
    clsW1 = din("clsW1", [H, L * H]); clsW2 = din("clsW2", [H, L * H])
    clsb1 = din("clsb1", [L, H]); clsb2 = din("clsb2", [L, H])
    cls_scale = din("cls_scale", [L, P])
    mW1a = din("mW1a", [H, 4 * H]); mW1b = din("mW1b", [H, 4 * H])
    mb1 = din("mb1", [H, K]); mW2r = din("mW2r", [H, 4 * K])
    mb2 = din("mb2", [K, 1])
    headW1 = din("headW1", [H, H]); headb1 = din("headb1", [H, 1])
    headW2 = din("headW2", [H, C]); headb2 = din("headb2", [C, 1])
    sel = din("sel", [P, 4])

    out_eon = nc.dram_tensor("out_eon", [SPC, 4], F32, kind="ExternalOutput")
    out_nw = nc.dram_tensor("out_nw", [BPC * P, 4], F32, kind="ExternalOutput")
    out_hstab = nc.dram_tensor("out_hstab", [B, H], F32, kind="ExternalOutput")
    out_logitsT = nc.dram_tensor("out_logitsT", [C, B], F32, kind="ExternalOutput")
    out_horig = nc.dram_tensor("out_horig", [B, H], F32, kind="ExternalOutput")

    tabA = [nc.dram_tensor(f"tabA{l}", [NPAD, F], F32, kind="Internal", addr_space="Shared")
            for l in range(L)]
    eon_tab = nc.dram_tensor("eon_tab", [NSLOT, 4], F32, kind="Internal", addr_space="Shared")
    nw_tab = nc.dram_tensor("nw_tab", [NPAD, 4], F32, kind="Internal", addr_space="Shared")
    maskx = nc.dram_tensor("maskx", [NPAD, F], F32, kind="Internal")
    tabC = [nc.dram_tensor(f"tabC{l}", [NPAD, F], F32, kind="Internal")
            for l in range(L - 1)]
    ag_in = [nc.dram_tensor(f"ag_in{l}", [BPC * P, F], F32, kind="Internal")
             for l in range(L)]
    eon_in = nc.dram_tensor("eon_in", [SPC, 4], F32, kind="Internal")
    nw_in = nc.dram_tensor("nw_in", [BPC * P, 4], F32, kind="Internal")
    agc_in = [nc.dram_tensor(f"agc_in{l}", [BPH * P, F], F32, kind="Internal")
              for l in range(L - 1)]
    pool_in = nc.dram_tensor("pool_in", [B, H], F32, kind="Internal")
    pool_out = nc.dram_tensor("pool_out", [B, H], F32, kind="Internal")

    with tile.TileContext(nc) as tc:
        with tc.tile_pool(name="const", bufs=1) as cp, \
             tc.tile_pool(name="sb", bufs=6) as sb, \
             tc.tile_pool(name="sb2", bufs=3) as sb2, \
             tc.tile_pool(name="msg", bufs=10) as msgp, \
             tc.tile_pool(name="oh", bufs=8) as ohp, \
             tc.tile_pool(name="bsb", bufs=3) as bsb, \
             tc.tile_pool(name="wp", bufs=1) as wp, \
             tc.tile_pool(name="psagg", bufs=2, space="PSUM") as psagg, \
             tc.tile_pool(name="ps1", bufs=2, space="PSUM") as ps1, \
             tc.tile_pool(name="ps2", bufs=2, space="PSUM") as ps2, \
             tc.tile_pool(name="ps3", bufs=1, space="PSUM") as ps3, \
             tc.tile_pool(name="ps4", bufs=1, space="PSUM") as ps4:

            ident = cp.tile([P, P], F32)
            make_identity(nc, ident[:])
            iota_i = cp.tile([P, P], I32)
            nc.gpsimd.iota(iota_i[:], pattern=[[1, P]], base=0,
                           channel_multiplier=0)
            iota_f = cp.tile([P, P], F32)
            nc.vector.tensor_copy(iota_f[:], iota_i[:])
            eps20_c = cp.tile([P, 1], F32)
            nc.vector.memset(eps20_c[:], EPS20)

            def load_w(name, dram, r, c_):
                w = wp.tile([r, c_], F32, tag=name)
                nc.sync.dma_start(out=w[:], in_=dram[:])
                return w

            encW1_s = load_w("encW1", encW1, H, L * H)
            encW2_s = load_w("encW2", encW2, H, L * H)
            clsW1_s = load_w("clsW1", clsW1, H, L * H)
            clsW2_s = load_w("clsW2", clsW2, H, L * H)
            mW1a_s = load_w("mW1a", mW1a, H, 4 * H)
            mW1b_s = load_w("mW1b", mW1b, H, 4 * H)
            mW2r_s = load_w("mW2r", mW2r, H, 4 * K)
            mb1_s = load_w("mb1", mb1, H, K)
            mb2_s = load_w("mb2", mb2, K, 1)
            hW1_s = load_w("headW1", headW1, H, H)
            hW2_s = load_w("headW2", headW2, H, C)
            hb1_s = load_w("headb1", headb1, H, 1)
            hb2_s = load_w("headb2", headb2, C, 1)
            sel_s = load_w("sel", sel, P, 4)
            invc_s = load_w("invc", inv_cnt_col, B, 1)

            def rows_to_cols(dram, r, tagn):
                """[r<=128, P] dram -> transposed [P, r] sbuf tile."""
                pad = sb.tile([P, P], F32, tag="r2c_pad")
                nc.vector.memset(pad[:], 0.0)
                nc.sync.dma_start(out=pad[:r, :], in_=dram[:])
                pst = ps1.tile([P, P], F32, tag="t1")
                nc.tensor.transpose(pst[:], pad[:], ident[:])
                cols = cp.tile([P, P], F32, tag=tagn)
                nc.vector.tensor_copy(cols[:], pst[:])
                return cols

            encb1_c = rows_to_cols(encb1, L, "c_eb1")   # [H, L] columns
            encb2_c = rows_to_cols(encb2, L, "c_eb2")
            clsb1_c = rows_to_cols(clsb1, L, "c_cb1")
            clsb2_c = rows_to_cols(clsb2, L, "c_cb2")
            encs_c = rows_to_cols(enc_scale, L, "c_es")
            clss_c = rows_to_cols(cls_scale, L, "c_cs")

            # ---------------- GIN layer ----------------
            def gin_layer(src_ap, dstl_ap, own_ap, table, nblocks, W1s, W2s,
                          b1c, b2c, sc_c, l, out_dram, eon_col=None,
                          eon_mask=None, h3_sink=None):
                for bi in range(nblocks):
                    srcs_t = sb.tile([P, Tmax], I32, tag="srcs")
                    nc.sync.dma_start(out=srcs_t[:],
                                      in_=src_ap[bi * P:(bi + 1) * P, :])
                    dl_t = sb.tile([P, Tmax], F32, tag="dls")
                    nc.sync.dma_start(out=dl_t[:],
                                      in_=dstl_ap[bi * P:(bi + 1) * P, :])
                    if eon_col is not None:
                        dlm = sb.tile([P, Tmax], F32, tag="dlm")
                        nc.vector.memset(dlm[:], -1.0)
                        nc.vector.copy_predicated(
                            out=dlm[:],
                            mask=eon_mask[:, bi * Tmax:(bi + 1) * Tmax],
                            data=dl_t[:])
                        dl_t = dlm
                    agg = psagg.tile([P, F], F32, tag="agg")
                    for t in range(Tmax):
                        msg = msgp.tile([P, F], F32, tag="msg")
                        nc.gpsimd.indirect_dma_start(
                            out=msg[:], out_offset=None, in_=table[:],
                            in_offset=bass.IndirectOffsetOnAxis(
                                ap=srcs_t[:, t:t + 1], axis=0))
                        oh = ohp.tile([P, P], F32, tag="oh")
                        nc.vector.tensor_tensor(
                            out=oh[:], in0=dl_t[:, t:t + 1].to_broadcast([P, P]),
                            in1=iota_f[:], op=OP.is_equal)
                        nc.tensor.matmul(agg[:], lhsT=oh[:], rhs=msg[:],
                                         start=(t == 0), stop=(t == Tmax - 1))
                    oidx = sb.tile([P, 1], I32, tag="oidx")
                    nc.sync.dma_start(out=oidx[:],
                                      in_=own_ap[bi * P:(bi + 1) * P, :])
                    hown = sb2.tile([P, F], F32, tag="hown")
                    nc.gpsimd.indirect_dma_start(
                        out=hown[:], out_offset=None, in_=table[:],
                        in_offset=bass.IndirectOffsetOnAxis(ap=oidx[:, :1], axis=0))
                    z = sb2.tile([P, F], F32, tag="z")
                    nc.scalar.activation(out=z[:], in_=hown[:], func=AF.Copy,
                                         scale=sc_c[:, l:l + 1])
                    nc.vector.tensor_tensor(out=z[:], in0=z[:], in1=agg[:],
                                            op=OP.add)
                    zt_ps = ps1.tile([P, P], F32, tag="t1")
                    nc.tensor.transpose(zt_ps[:], z[:], ident[:])
                    zt = sb2.tile([P, P], F32, tag="zt")
                    nc.vector.tensor_copy(zt[:], zt_ps[:])
                    h1_ps = ps2.tile([P, P], F32, tag="t2")
                    nc.tensor.matmul(h1_ps[:], lhsT=W1s[:, l * H:(l + 1) * H],
                                     rhs=zt[:], start=True, stop=True)
                    h1 = sb2.tile([P, P], F32, tag="h1")
                    nc.scalar.activation(out=h1[:], in_=h1_ps[:], func=AF.Relu,
                                         bias=b1c[:, l:l + 1])
                    h2_ps = ps3.tile([P, P], F32, tag="t3")
                    nc.tensor.matmul(h2_ps[:], lhsT=W2s[:, l * H:(l + 1) * H],
                                     rhs=h1[:], start=True, stop=True)
                    h2t = sb2.tile([P, P], F32, tag="h2t")
                    nc.scalar.activation(out=h2t[:], in_=h2_ps[:], func=AF.Relu,
                                         bias=b2c[:, l:l + 1])
                    hn_ps = ps4.tile([P, P], F32, tag="t4")
                    nc.tensor.transpose(hn_ps[:], h2t[:], ident[:])
                    hn = sb2.tile([P, P], F32, tag="hn")
                    nc.vector.tensor_copy(hn[:], hn_ps[:])
                    if h3_sink is not None:
                        h3_sink(bi, hn)
                    else:
                        nc.gpsimd.dma_start(
                            out=out_dram[bi * P:(bi + 1) * P, :], in_=hn[:])

            # ---------------- Phase A ----------------
            tabs_a = [x_t, tabA[0], tabA[1]]
            for l in range(L):
                gin_layer(src_blk, dstl_blk, own_nodes, tabs_a[l], BPC,
                          encW1_s, encW2_s, encb1_c, encb2_c, encs_c, l,
                          ag_in[l])
                nc.gpsimd.collective_compute(
                    "AllGather", OP.bypass, replica_groups=G8,
                    ins=[ag_in[l][:]], outs=[tabA[l][:]])

            Zt = tabA[2]

            # ---------------- Phase B ----------------
            CH = 512
            nch = SPC // CH
            for ci in range(nch):
                s0 = ci * CH
                zsT = sb.tile([P, CH], F32, tag="zsT")
                zdT = sb.tile([P, CH], F32, tag="zdT")
                for half_, (idx_dram, zT) in enumerate([(srcB, zsT), (dstB, zdT)]):
                    tp = ps1.tile([P, CH], F32, tag="t1")
                    for j in range(CH // P):
                        it = sb.tile([P, 1], I32, tag="bidx")
                        nc.sync.dma_start(
                            out=it[:], in_=idx_dram[s0 + j * P:s0 + (j + 1) * P, :])
                        zg = msgp.tile([P, F], F32, tag="msg")
                        nc.gpsimd.indirect_dma_start(
                            out=zg[:], out_offset=None, in_=Zt[:],
                            in_offset=bass.IndirectOffsetOnAxis(ap=it[:, :1], axis=0))
                        nc.tensor.transpose(tp[:, j * P:(j + 1) * P], zg[:],
                                            ident[:])
                    nc.vector.tensor_copy(zT[:], tp[:])
                eonT = sb.tile([4, CH], F32, tag="eonT")
                for hs in range(4):
                    h1_ps = ps2.tile([P, CH], F32, tag="t2")
                    nc.tensor.matmul(h1_ps[:], lhsT=mW1a_s[:, hs * H:(hs + 1) * H],
                                     rhs=zsT[:], start=True, stop=False)
                    nc.tensor.matmul(h1_ps[:], lhsT=mW1b_s[:, hs * H:(hs + 1) * H],
                                     rhs=zdT[:], start=False, stop=True)
                    h1b = bsb.tile([P, CH], F32, tag="h1b")
                    nc.scalar.activation(out=h1b[:], in_=h1_ps[:], func=AF.Relu,
                                         bias=mb1_s[:, hs:hs + 1])
                    em_ps = ps3.tile([4, CH], F32, tag="t3")
                    nc.tensor.matmul(em_ps[:], lhsT=mW2r_s[:, hs * 4:(hs + 1) * 4],
                                     rhs=h1b[:], start=(hs == 0), stop=(hs == 3))
                em = sb.tile([4, CH], F32, tag="em")
                nc.vector.tensor_copy(em[:], em_ps[:])
                nc.vector.tensor_tensor(out=em[:], in0=em[:],
                                        in1=mb2_s[:, :1].to_broadcast([4, CH]),
                                        op=OP.add)
                ut = sb.tile([4, CH], F32, tag="ut")
                nc.sync.dma_start(out=ut[:], in_=uT[:, s0:s0 + CH])
                t1 = sb.tile([4, CH], F32, tag="gt1")
                nc.scalar.activation(out=t1[:], in_=ut[:], func=AF.Ln,
                                     bias=eps20_c[:4, :1])
                t2 = sb.tile([4, CH], F32, tag="gt2")
                nc.scalar.activation(out=t2[:], in_=t1[:], func=AF.Ln,
                                     bias=eps20_c[:4, :1], scale=-1.0)
                # e_on = (em + mb2 - t2 > 0)
                nc.vector.tensor_tensor(out=em[:], in0=em[:], in1=t2[:],
                                        op=OP.subtract)
                emb = sb.tile([4, CH], F32, tag="emb")
                nc.vector.tensor_scalar(out=emb[:], in0=em[:], scalar1=0.0,
                                        scalar2=None, op0=OP.is_gt)
                for j in range(CH // P):
                    ep = ps4.tile([P, 4], F32, tag="t4")
                    nc.tensor.transpose(ep[:], emb[:, j * P:(j + 1) * P],
                                        ident[:4, :4])
                    er = sb.tile([P, 4], F32, tag="er")
                    nc.vector.tensor_copy(er[:], ep[:, :4])
                    nc.gpsimd.dma_start(
                        out=eon_in[s0 + j * P:s0 + (j + 1) * P, :], in_=er[:])
            nc.gpsimd.collective_compute(
                "AllGather", OP.bypass, replica_groups=G8,
                ins=[eon_in[:]], outs=[eon_tab[:]])
            nc.sync.dma_start(out=out_eon[:], in_=eon_in[:])

            # ---------------- nw ----------------
            for bi in range(BPC):
                eids_t = sb.tile([P, Tnw], I32, tag="eids")
                nc.sync.dma_start(out=eids_t[:],
                                  in_=eid_blk[bi * P:(bi + 1) * P, :])
                epl_t = sb.tile([P, Tnw], F32, tag="epls")
                nc.sync.dma_start(out=epl_t[:],
                                  in_=epl_blk[bi * P:(bi + 1) * P, :])
                cnt_ps = psagg.tile([P, 4], F32, tag="agg")
                for t in range(Tnw):
                    ev = msgp.tile([P, 4], F32, tag="ev")
                    nc.gpsimd.indirect_dma_start(
                        out=ev[:], out_offset=None, in_=eon_tab[:],
                        in_offset=bass.IndirectOffsetOnAxis(
                            ap=eids_t[:, t:t + 1], axis=0))
                    oh = ohp.tile([P, P], F32, tag="oh")
                    nc.vector.tensor_tensor(
                        out=oh[:], in0=epl_t[:, t:t + 1].to_broadcast([P, P]),
                        in1=iota_f[:], op=OP.is_equal)
                    nc.tensor.matmul(cnt_ps[:], lhsT=oh[:], rhs=ev[:],
                                     start=(t == 0), stop=(t == Tnw - 1))
                nwt = sb2.tile([P, 4], F32, tag="nwt")
                nc.vector.tensor_scalar(out=nwt[:], in0=cnt_ps[:], scalar1=0.0,
                                        scalar2=None, op0=OP.is_gt)
                nc.gpsimd.dma_start(out=nw_in[bi * P:(bi + 1) * P, :],
                                    in_=nwt[:])
            nc.gpsimd.collective_compute(
                "AllGather", OP.bypass, replica_groups=G8,
                ins=[nw_in[:]], outs=[nw_tab[:]])
            nc.sync.dma_start(out=out_nw[:], in_=nw_in[:])

            # ---------------- masked_x ----------------
            for gb in range(NBLK):
                xr = sb2.tile([P, F], F32, tag="xr")
                nc.sync.dma_start(out=xr[:], in_=x_t[gb * P:(gb + 1) * P, :])
                nwr = sb.tile([P, 4], F32, tag="nwr")
                nc.sync.dma_start(out=nwr[:], in_=nw_tab[gb * P:(gb + 1) * P, :])
                nc.vector.tensor_tensor(out=nwr[:], in0=nwr[:], in1=sel_s[:],
                                        op=OP.mult)
                nwk = sb.tile([P, 1], F32, tag="nwk")
                nc.vector.tensor_reduce(nwk[:], nwr[:], mybir.AxisListType.X, OP.add)
                mx = sb2.tile([P, F], F32, tag="mx")
                nc.scalar.activation(out=mx[:], in_=xr[:], func=AF.Copy,
                                     scale=nwk[:, :1])
                nc.gpsimd.dma_start(out=maskx[gb * P:(gb + 1) * P, :], in_=mx[:])

            # ---------------- e_on column for my half ----------------
            NEK = HS // P
            eonk = cp.tile([P, NEK], F32)
            for i in range(NEK):
                ci_t = sb.tile([P, 1], I32, tag="cid")
                nc.sync.dma_start(out=ci_t[:], in_=cslot[i * P:(i + 1) * P, :])
                ev = msgp.tile([P, 4], F32, tag="ev")
                nc.gpsimd.indirect_dma_start(
                    out=ev[:], out_offset=None, in_=eon_tab[:],
                    in_offset=bass.IndirectOffsetOnAxis(ap=ci_t[:, :1], axis=0))
                nc.vector.tensor_tensor(out=ev[:], in0=ev[:], in1=sel_s[:],
                                        op=OP.mult)
                nc.vector.tensor_reduce(eonk[:, i:i + 1], ev[:], mybir.AxisListType.X, OP.add)
            eonki = cp.tile([P, NEK], mybir.dt.int8, tag="eonki")
            nc.vector.tensor_copy(eonki[:], eonk[:])

            # ---------------- Phase C ----------------
            hacc = cp.tile([B, H], F32)
            nc.vector.memset(hacc[:], 0.0)

            def pool_sink(bi, hn):
                bl = sb.tile([P, 1], F32, tag="bl")
                nc.sync.dma_start(out=bl[:],
                                  in_=batchl_half[bi * P:(bi + 1) * P, :])
                ohb = ohp.tile([P, B], F32, tag="ohb")
                nc.vector.tensor_tensor(
                    out=ohb[:], in0=bl[:, :1].to_broadcast([P, B]),
                    in1=iota_f[:, :B], op=OP.is_equal)
                pps = ps3.tile([B, H], F32, tag="t3")
                nc.tensor.matmul(pps[:], lhsT=ohb[:], rhs=hn[:], start=True,
                                 stop=True)
                nc.vector.tensor_tensor(out=hacc[:], in0=hacc[:], in1=pps[:],
                                        op=OP.add)

            tabs_c = [maskx, tabC[0], tabC[1]]
            for l in range(L):
                sink = pool_sink if l == L - 1 else None
                outd = agc_in[l] if l < L - 1 else None
                gin_layer(src_half, dstl_half, own_half, tabs_c[l], BPH,
                          clsW1_s, clsW2_s, clsb1_c, clsb2_c, clss_c, l,
                          outd, eon_col=eonk, eon_mask=eonki, h3_sink=sink)
                if l < L - 1:
                    nc.gpsimd.collective_compute(
                        "AllGather", OP.bypass, replica_groups=G2,
                        ins=[agc_in[l][:]], outs=[tabC[l][:]])

            nc.gpsimd.dma_start(out=pool_in[:], in_=hacc[:])
            nc.gpsimd.collective_compute(
                "AllReduce", OP.add, replica_groups=G2,
                ins=[pool_in[:]], outs=[pool_out[:]])
            hs_t = sb.tile([B, H], F32, tag="hs")
            nc.sync.dma_start(out=hs_t[:], in_=pool_out[:])
            nc.scalar.activation(out=hs_t[:], in_=hs_t[:], func=AF.Copy,
                                 scale=invc_s[:, :1])
            nc.gpsimd.dma_start(out=out_hstab[:], in_=hs_t[:])

            # head MLP
            def head(hs_tile, Wa, ba, Wb, bb, out_dram):
                pad = sb.tile([P, P], F32, tag="hpad")
                nc.vector.memset(pad[:], 0.0)
                nc.vector.tensor_copy(pad[:B, :], hs_tile[:])
                hsT_ps = ps1.tile([P, P], F32, tag="t1")
                nc.tensor.transpose(hsT_ps[:], pad[:], ident[:])
                hsT = sb.tile([P, B], F32, tag="hsT")
                nc.vector.tensor_copy(hsT[:], hsT_ps[:, :B])
                m1 = ps2.tile([P, B], F32, tag="t2")
                nc.tensor.matmul(m1[:], lhsT=Wa[:], rhs=hsT[:], start=True,
                                 stop=True)
                r1 = sb.tile([P, B], F32, tag="hr1")
                nc.scalar.activation(out=r1[:], in_=m1[:], func=AF.Relu,
                                     bias=ba[:, :1])
                m2 = ps4.tile([C, B], F32, tag="t4")
                nc.tensor.matmul(m2[:], lhsT=Wb[:], rhs=r1[:], start=True,
                                 stop=True)
                lg = sb.tile([C, B], F32, tag="lg")
                nc.scalar.activation(out=lg[:], in_=m2[:], func=AF.Copy)
                nc.vector.tensor_tensor(out=lg[:], in0=lg[:],
                                        in1=bb[:, :1].to_broadcast([C, B]),
                                        op=OP.add)
                nc.gpsimd.dma_start(out=out_dram[:], in_=lg[:])

            head(hs_t, hW1_s, hb1_s, hW2_s, hb2_s, out_logitsT)

            # h_orig from Z (all cores redundantly)
            hacc2 = cp.tile([B, H], F32)
            nc.vector.memset(hacc2[:], 0.0)
            for gb in range(NBLK):
                hr = sb2.tile([P, F], F32, tag="hr")
                nc.sync.dma_start(out=hr[:], in_=Zt[gb * P:(gb + 1) * P, :])
                bl = sb.tile([P, 1], F32, tag="bl")
                nc.sync.dma_start(out=bl[:], in_=batchl[gb * P:(gb + 1) * P, :])
                ohb = ohp.tile([P, B], F32, tag="ohb")
                nc.vector.tensor_tensor(
                    out=ohb[:], in0=bl[:, :1].to_broadcast([P, B]),
                    in1=iota_f[:, :B], op=OP.is_equal)
                pps = ps3.tile([B, H], F32, tag="t3")
                nc.tensor.matmul(pps[:], lhsT=ohb[:], rhs=hr[:], start=True,
                                 stop=True)
                nc.vector.tensor_tensor(out=hacc2[:], in0=hacc2[:], in1=pps[:],
                                        op=OP.add)
            ho = sb.tile([B, H], F32, tag="ho")
            nc.scalar.activation(out=ho[:], in_=hacc2[:], func=AF.Copy,
                                 scale=invc_s[:, :1])
            nc.gpsimd.dma_start(out=out_horig[:], in_=ho[:])

    nc.compile()
    return nc


_CACHE = {}


def kernel(**inputs):
    pp = _host_prep(inputs)
    Tmax, Tnw = pp["Tmax"], pp["Tnw"]
    key = (Tmax, Tnw)
    if key not in _CACHE:
        _CACHE[key] = _build(Tmax, Tnw)
    nc = _CACHE[key]
    in_maps = _per_core_inputs(pp, inputs)
    import os
    tr = bool(os.environ.get("KERNEL_TRACE"))
    res = bass_utils.run_bass_kernel_spmd(nc, in_maps, core_ids=list(range(NC8)),
                                          trace=tr)
    global LAST_RESULT
    LAST_RESULT = res
    rs = res.results

    SPC = BPC * Tmax * P
    eon_full = np.concatenate([rs[c]["out_eon"] for c in range(NC8)], axis=0)
    nw_full = np.concatenate([rs[c]["out_nw"] for c in range(NC8)], axis=0)
    edge_masks = eon_full[pp["slot_of_edge"]][:, :, None].astype(np.float32)
    node_masks = nw_full[:N][:, :, None].astype(np.float32)
    expert_logits = np.stack([rs[2 * k]["out_logitsT"].T for k in range(K)],
                             axis=1).astype(np.float32)
    h_stable_all = np.stack([rs[2 * k]["out_hstab"] for k in range(K)],
                            axis=1).astype(np.float32)
    h_orig = rs[0]["out_horig"].astype(np.float32)
    return (expert_logits, h_stable_all, h_orig, node_masks, edge_masks)
